# revision 16
# baseline (speedup 1.0000x reference)
"""Causal self-attention block (qkv proj -> causal MHA -> out proj) on 8 TRN2
NeuronCores.

Sharding: core c -> (batch b = c//2, head-group g = c%2). Each core computes
its batch's attention for 8 of the 16 heads (Megatron column-parallel qkv,
row-parallel out-proj), then a chunked pairwise ReduceScatter; each core
returns one half of its batch's rows and the host reassembles.

Pipeline structure (single TileContext, fully static):
  for qt in 0..3:
    attention for all head pairs on query tile qt (ACT-exp paced), with the
    NEXT t-tile's qkv-projection matmul work interleaved between attention
    slots to keep the PE dense; then the out-projection + ReduceScatter for
    the finished row chunk, which overlaps the next iteration.

Kernel-level choices:
  - Q^T/K^T produced directly in [c', t] orientation (w_qkv chunks stationary,
    PE-transposed x^T moving); head pairs packed 64+64 into 128-partition
    tiles so both heads' QK^T matmuls run concurrently via PE row tiling,
    writing two halves of one 2-bank PSUM tile; mask-add and exp are then one
    instruction per slot over both heads.
  - Diagonal blocks are column-trimmed to the causally-visible range.
  - V carries an all-ones column per head so the A@V matmul (M=65) emits
    softmax row-sums for free; normalization uses a fast approximate
    reciprocal and a partition-broadcast done by DMA (no PE/ACT cost).
  - All matmul operands are bf16 (fp32 accumulation in PSUM).
"""

import numpy as np
import ml_dtypes

import concourse.bass as bass
import concourse.bacc as bacc
import concourse.mybir as mybir
import concourse.tile as tile
from concourse.bass_utils import run_bass_kernel_spmd
from concourse.masks import make_identity
from contextlib import ExitStack

F32 = mybir.dt.float32
BF16 = mybir.dt.bfloat16
P = 128
D = 64
BF16NP = ml_dtypes.bfloat16


def build_program(T, C, HC, num_devices, groups):
    NPAIR = HC // 2
    CT = C // P            # contraction chunks of the qkv matmul
    NT5 = T // 512         # 512-wide t tiles (also RS chunks)
    NTS = T // P           # 128-wide t subtiles
    CSH = HC * D           # local c-tilde width (V / Y^T columns)
    NCO = C // 512         # 512-wide out-column tiles
    QKW = 2 * HC * D       # q+k c' columns
    NQK = QKW // P         # qk c'-tiles (2 per head pair)
    SCALE = 1.0 / np.sqrt(np.float32(D))
    NEG = -1.0e9
    HROWS = T // (2 * NT5)  # per-core rows of one RS chunk

    nc = bacc.Bacc("TRN2", target_bir_lowering=False, debug=False,
                   num_devices=num_devices)
    xbf = nc.dram_tensor("xbf", [T, C], BF16, kind="ExternalInput").ap()
    wqk = nc.dram_tensor("wqk", [C, QKW], BF16, kind="ExternalInput").ap()
    wv = nc.dram_tensor("wv", [C, CSH], BF16, kind="ExternalInput").ap()
    bqk = nc.dram_tensor("bqk", [P, NQK], F32, kind="ExternalInput").ap()
    bvb = nc.dram_tensor("bvb", [P, CSH], F32, kind="ExternalInput").ap()
    bprb = nc.dram_tensor("bprb", [P, C], F32, kind="ExternalInput").ap()
    wpr = nc.dram_tensor("wpr", [CSH, C], BF16, kind="ExternalInput").ap()
    out = nc.dram_tensor("out", [T // 2, C], F32, kind="ExternalOutput").ap()

    ADD = mybir.AluOpType.add
    MULT = mybir.AluOpType.mult
    EXP = mybir.ActivationFunctionType.Exp

    with tile.TileContext(nc) as tc, ExitStack() as ctx:
        # ---- pools -----------------------------------------------------
        const = ctx.enter_context(tc.tile_pool(name="const", bufs=1))
        bqk_sb = const.tile([P, NQK], F32, name="bqk_sb")
        nc.sync.dma_start(out=bqk_sb[:], in_=bqk[:])
        bvb_sb = const.tile([P, CSH], F32, name="bvb_sb")
        nc.sync.dma_start(out=bvb_sb[:], in_=bvb[:])
        bprb_sb = const.tile([P, C], F32, name="bprb_sb")
        nc.sync.dma_start(out=bprb_sb[:], in_=bprb[:])
        ones_bf = const.tile([1, 64], BF16, name="ones_bf")
        nc.vector.memset(ones_bf[:], 1.0)
        masks = []
        for i in range(4):
            mk = const.tile([P, 512], F32, name=f"mask{i}")
            nc.gpsimd.memset(mk[:], 0.0)
            nc.gpsimd.affine_select(
                out=mk[:], in_=mk[:], compare_op=mybir.AluOpType.is_ge,
                fill=NEG, base=-(P * i), pattern=[[1, 512]],
                channel_multiplier=-1)
            masks.append(mk)

        qk_pool = ctx.enter_context(tc.tile_pool(name="qkp", bufs=1))
        QK_sb = [qk_pool.tile([P, T], BF16, name=f"qk{ct}") for ct in range(NQK)]
        v_pool = ctx.enter_context(tc.tile_pool(name="vp", bufs=1))
        V_sb = [v_pool.tile([P, HC * 65], BF16, name=f"v{i}") for i in range(NTS)]
        yn_pool = ctx.enter_context(tc.tile_pool(name="ynp", bufs=1))
        Yn_sb = [yn_pool.tile([P, T], BF16, name=f"yn{p}") for p in range(NPAIR)]
        wpr_pool = ctx.enter_context(tc.tile_pool(name="wprp", bufs=1))
        wpr_sb = [wpr_pool.tile([P, C], BF16, name=f"wpr{p}") for p in range(NPAIR)]
        for p in range(NPAIR):
            nc.sync.dma_start(out=wpr_sb[p][:], in_=wpr[p * P:(p + 1) * P, :])
        wqk_pool = ctx.enter_context(tc.tile_pool(name="wqkp", bufs=1))
        wqk_sb = [wqk_pool.tile([P, QKW], BF16, name=f"wqk{j}") for j in range(CT)]
        for j in range(CT):
            nc.sync.dma_start(out=wqk_sb[j][:], in_=wqk[j * P:(j + 1) * P, :])
        wv_pool = ctx.enter_context(tc.tile_pool(name="wvp", bufs=1))
        wv_sb = [wv_pool.tile([P, CSH], BF16, name=f"wv{j}") for j in range(CT)]
        for j in range(CT):
            nc.sync.dma_start(out=wv_sb[j][:], in_=wv[j * P:(j + 1) * P, :])

        dram = ctx.enter_context(tc.tile_pool(name="dram", bufs=1, space="DRAM"))
        ob = [dram.tile([512, C], F32, name=f"ob{c}") for c in range(NT5)]
        orh = [dram.tile([HROWS, C], F32, name=f"orh{c}") for c in range(NT5)]

        xt_pool = ctx.enter_context(tc.tile_pool(name="xtp", bufs=2 * CT))
        exp_pool = ctx.enter_context(tc.tile_pool(name="expp", bufs=4))
        yc_pool = ctx.enter_context(tc.tile_pool(name="ycp", bufs=2))
        r_pool = ctx.enter_context(tc.tile_pool(name="rp", bufs=6))
        ynb_pool = ctx.enter_context(tc.tile_pool(name="ynbp", bufs=2))
        oc_pool = ctx.enter_context(tc.tile_pool(name="ocp", bufs=4))

        big = ctx.enter_context(tc.tile_pool(name="big", bufs=2, space="PSUM"))
        ps = ctx.enter_context(tc.tile_pool(name="ps", bufs=2, space="PSUM"))
        psY = ctx.enter_context(tc.tile_pool(name="psY", bufs=1, space="PSUM"))

        # ---- stage A emission units (qkv projection for t-tile tt) -----
        def stage_a_units(tt):
            units = []
            xts = []

            def load_xt():
                for j in range(CT):
                    xt = xt_pool.tile([P, 512], BF16, name="xt")
                    nc.sync.dma_start(
                        out=xt[:],
                        in_=xbf[tt * 512:(tt + 1) * 512, j * P:(j + 1) * P],
                        transpose=True)
                    xts.append(xt)
            units.append(load_xt)

            def qkt(ct):
                p_ = ps.tile([P, 512], F32, name="ps", tag="ps")
                for j in range(CT):
                    nc.tensor.matmul(
                        p_[:], wqk_sb[j][:, ct * P:(ct + 1) * P],
                        xts[j][:], start=(j == 0), stop=(j == CT - 1))
                nc.vector.tensor_scalar_add(
                    QK_sb[ct][:, tt * 512:(tt + 1) * 512], p_[:],
                    bqk_sb[:, ct:ct + 1])
            for ct in range(NQK):
                units.append(lambda ct=ct: qkt(ct))

            def vproj(i):
                p_ = ps.tile([P, CSH], F32, name="ps", tag="ps")
                for j in range(CT):
                    nc.tensor.matmul(
                        p_[:], xts[j][:, i * P:(i + 1) * P],
                        wv_sb[j][:], start=(j == 0), stop=(j == CT - 1))
                vt = V_sb[tt * 4 + i]
                vt3 = vt.rearrange("p (h e) -> p h e", e=65)
                nc.vector.tensor_tensor(
                    vt3[:, :, 0:64], p_.rearrange("p (h d) -> p h d", d=D),
                    bvb_sb.rearrange("p (h d) -> p h d", d=D), op=ADD)
                nc.vector.memset(vt3[:, :, 64:65], 1.0)
            for i in range(4):
                units.append(lambda i=i: vproj(i))
            return units

        # ---- stage B: attention for query tile qt, one head pair -------
        DEPTH = 2  # AV trails QK by this many slots on the PE

        def attention(p, qt, a_units, pend):
            qa = QK_sb[2 * p]
            ka = QK_sb[2 * p + 1]
            hA, hB = 2 * p, 2 * p + 1
            nkt = 4 * qt + 4
            q0 = qt * 512
            pyA = psY.tile([65, 512], F32, name="pyA")
            pyB = psY.tile([65, 512], F32, name="pyB")

            def emit_av(kt, eAB, off, pyA=pyA, pyB=pyB, hA=hA, hB=hB, nkt=nkt):
                vt = V_sb[kt]
                nc.tensor.matmul(
                    pyA[:, off:512], vt[:, hA * 65:(hA + 1) * 65],
                    eAB[:, off:512],
                    start=(kt == 0), stop=(kt == nkt - 1))
                nc.tensor.matmul(
                    pyB[:, off:512], vt[:, hB * 65:(hB + 1) * 65],
                    eAB[:, 512 + off:1024],
                    start=(kt == 0), stop=(kt == nkt - 1))

            for kt in range(nkt):
                di = kt - 4 * qt
                off = P * di if di > 0 else 0
                L = 512 - off
                st = big.tile([P, 1024], F32, name="stAB")
                nc.tensor.matmul(
                    st[:, off:512], ka[0:64, kt * P:(kt + 1) * P],
                    qa[0:64, q0 + off:q0 + 512])
                nc.tensor.matmul(
                    st[:, 512 + off:1024], ka[64:128, kt * P:(kt + 1) * P],
                    qa[64:128, q0 + off:q0 + 512], tile_position=(64, 0))
                st3 = st.rearrange("p (two n) -> p two n", two=2)[:, :, off:512]
                if di >= 0:
                    mk = masks[di][:, None, off:512].broadcast_to([P, 2, L])
                    nc.vector.tensor_tensor(st3, st3, mk, op=ADD)
                eAB = exp_pool.tile([P, 1024], BF16, name="eAB")
                e3 = eAB.rearrange("p (two n) -> p two n", two=2)[:, :, off:512]
                nc.scalar.activation(e3, st3, EXP, scale=SCALE)
                pend.append(lambda kt=kt, eAB=eAB, off=off: emit_av(kt, eAB, off))
                if a_units:
                    a_units.pop(0)()
                while len(pend) > DEPTH:
                    pend.pop(0)()

            def normalize(pyA=pyA, pyB=pyB, p=p, q0=q0):
                # Y[:, q] /= rowsum[q]
                rrA = r_pool.tile([1, 512], F32, name="rrA", tag="rw")
                nc.scalar.copy(rrA[:], pyA[64:65, :])
                rrB = r_pool.tile([1, 512], F32, name="rrB", tag="rw")
                nc.scalar.copy(rrB[:], pyB[64:65, :])
                ycA = yc_pool.tile([64, 512], F32, name="ycA")
                nc.vector.tensor_copy(ycA[:], pyA[0:64, :])
                ycB = yc_pool.tile([64, 512], F32, name="ycB")
                nc.vector.tensor_copy(ycB[:], pyB[0:64, :])
                rA = r_pool.tile([1, 512], F32, name="rA", tag="rw")
                nc.vector.reciprocal_approx_fast(rA[:], rrA[:])
                rB = r_pool.tile([1, 512], F32, name="rB", tag="rw")
                nc.vector.reciprocal_approx_fast(rB[:], rrB[:])
                rAb = r_pool.tile([1, 512], BF16, name="rAb", tag="rw")
                nc.vector.tensor_copy(rAb[:], rA[:])
                rBb = r_pool.tile([1, 512], BF16, name="rBb", tag="rw")
                nc.vector.tensor_copy(rBb[:], rB[:])
                bcA = ps.tile([64, 512], F32, name="ps", tag="ps")
                nc.tensor.matmul(bcA[:], ones_bf[:], rAb[:])
                bcB = ps.tile([64, 512], F32, name="ps", tag="ps")
                nc.tensor.matmul(bcB[:], ones_bf[:], rBb[:])
                nc.vector.tensor_tensor(
                    Yn_sb[p][0:64, q0:q0 + 512], ycA[:], bcA[:], op=MULT)
                ynB = ynb_pool.tile([64, 512], BF16, name="ynB")
                nc.vector.tensor_tensor(ynB[:], ycB[:], bcB[:], op=MULT)
                nc.sync.dma_start(out=Yn_sb[p][64:128, q0:q0 + 512], in_=ynB[:])
            pend.append(normalize)

        # ---- stage C: out projection + ReduceScatter for chunk qt ------
        def proj_units(qt):
            units = []

            def ts_unit(ts, co):
                po = ps.tile([P, 512], F32, name="ps", tag="ps")
                for p in range(NPAIR):
                    nc.tensor.matmul(
                        po[:], Yn_sb[p][:, ts * P:(ts + 1) * P],
                        wpr_sb[p][:, co * 512:(co + 1) * 512],
                        start=(p == 0), stop=(p == NPAIR - 1))
                oc = oc_pool.tile([P, 512], F32, name="oc")
                nc.vector.tensor_tensor(
                    oc[:], po[:], bprb_sb[:, co * 512:(co + 1) * 512], op=ADD)
                nc.sync.dma_start(
                    out=ob[qt][(ts - 4 * qt) * P:(ts - 4 * qt + 1) * P,
                               co * 512:(co + 1) * 512],
                    in_=oc[:])
            for ts in range(4 * qt, 4 * qt + 4):
                for co in range(NCO):
                    units.append(lambda ts=ts, co=co: ts_unit(ts, co))

            def rs_unit():
                nc.gpsimd.collective_compute(
                    "ReduceScatter", mybir.AluOpType.add, replica_groups=groups,
                    ins=[ob[qt].opt()], outs=[orh[qt].opt()])
                nc.sync.dma_start(
                    out=out[qt * HROWS:(qt + 1) * HROWS, :], in_=orh[qt][:])
            units.append(rs_unit)
            return units

        # ---- top-level pipeline ---------------------------------------
        # Process query tiles in order [1, 2, 3, 0]; fillers keep the PE
        # dense during the ACT-paced attention slots: the next t-tile's qkv
        # projection and the previous chunk's out-projection + RS.
        pend = []
        if NT5 == 4:
            for u in stage_a_units(0) + stage_a_units(1):
                u()
            fill = {1: stage_a_units(2),
                    2: stage_a_units(3),
                    3: proj_units(1) + proj_units(2),
                    0: proj_units(3)}
            order = [1, 2, 3, 0]
            for qt in order:
                # proj fillers read Yn written by the previous qt's tail
                # normalizes -- flush them before consuming new fillers
                while pend:
                    pend.pop(0)()
                fillers = fill[qt]
                for p in range(NPAIR):
                    attention(p, qt, fillers, pend)
                for u in fillers:
                    u()
            while pend:
                pend.pop(0)()
            for u in proj_units(0):
                u()
        else:  # small test configs: plain order
            fillers = []
            for u in stage_a_units(0):
                u()
            for qt in range(NT5):
                for u in fillers:
                    u()
                if qt + 1 < NT5:
                    for u in stage_a_units(qt + 1):
                        u()
                for p in range(NPAIR):
                    attention(p, qt, [], pend)
                while pend:
                    pend.pop(0)()
                fillers = proj_units(qt)
            for u in fillers:
                u()

    nc.compile()
    return nc


def prep_core_inputs(x, w_qkv, b_qkv, w_proj, b_proj, b, g, HC):
    """Host-side shard + layout permutation for core (batch b, head group g)."""
    C = x.shape[-1]
    heads = [g * HC + i for i in range(HC)]
    wq, wk, wvf = w_qkv[:, 0:C], w_qkv[:, C:2 * C], w_qkv[:, 2 * C:3 * C]
    bq, bk, bvf = b_qkv[0:C], b_qkv[C:2 * C], b_qkv[2 * C:3 * C]
    qk_cols, bqk_cols = [], []
    for p in range(HC // 2):
        h0, h1 = heads[2 * p], heads[2 * p + 1]
        qk_cols += [wq[:, h0 * D:(h0 + 1) * D], wq[:, h1 * D:(h1 + 1) * D]]
        bqk_cols += [bq[h0 * D:(h0 + 1) * D], bq[h1 * D:(h1 + 1) * D]]
        qk_cols += [wk[:, h0 * D:(h0 + 1) * D], wk[:, h1 * D:(h1 + 1) * D]]
        bqk_cols += [bk[h0 * D:(h0 + 1) * D], bk[h1 * D:(h1 + 1) * D]]
    wqk = np.concatenate(qk_cols, axis=1).astype(BF16NP)
    bqk = np.concatenate(bqk_cols).astype(np.float32)
    bqk = np.ascontiguousarray(bqk.reshape(-1, P).T)
    wv = np.concatenate(
        [wvf[:, h * D:(h + 1) * D] for h in heads], axis=1).astype(BF16NP)
    bv = np.concatenate([bvf[h * D:(h + 1) * D] for h in heads]).astype(np.float32)
    bvb = np.ascontiguousarray(np.broadcast_to(bv[None, :], (P, bv.size)))
    wpr = np.concatenate(
        [w_proj[h * D:(h + 1) * D, :] for h in heads], axis=0).astype(BF16NP)
    bpr = (b_proj / 2.0).astype(np.float32)
    bprb = np.ascontiguousarray(np.broadcast_to(bpr[None, :], (P, C)))
    return {
        "xbf": np.ascontiguousarray(x[b].astype(BF16NP)),
        "wqk": np.ascontiguousarray(wqk), "bqk": bqk,
        "wv": np.ascontiguousarray(wv), "bvb": bvb,
        "wpr": np.ascontiguousarray(wpr), "bprb": bprb,
    }


def assemble_output(results, B, T, C):
    """Interleave the per-core ReduceScatter halves back to [B, T, C]."""
    NT5 = T // 512
    HROWS = T // (2 * NT5)
    full = np.empty((B, T, C), np.float32)
    for b in range(B):
        for r in range(2):
            o = np.asarray(results[2 * b + r]["out"])
            for c in range(NT5):
                dst = c * 512 + r * HROWS
                full[b, dst:dst + HROWS] = o[c * HROWS:(c + 1) * HROWS]
    return full


_CACHE = {}


def kernel(x, w_qkv, b_qkv, w_proj, b_proj, _trace=False):
    x = np.asarray(x, np.float32)
    w_qkv = np.asarray(w_qkv, np.float32)
    b_qkv = np.asarray(b_qkv, np.float32)
    w_proj = np.asarray(w_proj, np.float32)
    b_proj = np.asarray(b_proj, np.float32)
    B, T, C = x.shape
    H = 16
    HC = H // 2
    groups = [[2 * b, 2 * b + 1] for b in range(B)]

    key = (T, C, HC, 2 * B)
    if key not in _CACHE:
        _CACHE[key] = build_program(T, C, HC, 2 * B, groups)
    nc = _CACHE[key]

    in_maps = []
    for c in range(2 * B):
        in_maps.append(
            prep_core_inputs(x, w_qkv, b_qkv, w_proj, b_proj, c // 2, c % 2, HC))
    res = run_bass_kernel_spmd(nc, in_maps, list(range(2 * B)), trace=_trace)
    full = assemble_output(res.results, B, T, C)
    if _trace:
        return full, res
    return full
